# revision 1
# baseline (speedup 1.0000x reference)
"""KGCN message-passing kernel for 8 TRN2 NeuronCores.

Data-parallel over the batch axis (1024 drugs -> 128 per core). Tables are
replicated per core. All indirect DMAs use the HW-verified [128,1]-index
pattern (one gathered row per partition per instruction, contiguous dest).

Relation embeddings are never gathered: scores come from S = sv @ rel_emb^T
on the TensorEngine, and per-neighbor softmax weights are selected from
exp(S) with a one-hot compare+reduce on the VectorEngine (exact).

Tables:
  TBL (int32 [200064, 96]): rows 0..199999 =
      [ent_emb[e] f32-bits | adj_ent[e] | adj_rel[e]]  (rows 200000+ zero)
  EMB (f32 [200064, 32]): ent_emb rows (+rel_emb rows appended, unused)

Hardcoded: NUM_ENT=200000, NUM_REL=64, DIM=32, N_NEIGHBOR=32, N_ITER=2,
BATCH=1024, 8 cores.
"""

import sys

import numpy as np

try:
    import concourse.bass as bass  # noqa: F401
except ImportError:  # harness env may not have concourse on sys.path
    sys.path.insert(0, "/opt/trn_rl_repo")

import concourse.bacc as bacc_mod
import concourse.mybir as mybir
from concourse.bass import IndirectOffsetOnAxis
from concourse.masks import make_identity
from concourse.tile import TileContext

NUM_ENT = 200000
NUM_REL = 64
DIM = 32
NNB = 32
BATCH = 1024
N_CORES = 8
B_CORE = BATCH // N_CORES  # 128 drugs per core
ROW_I32 = 3 * DIM  # 96 int32 per fused TBL row
TBL_ROWS = NUM_ENT + NUM_REL

FP = mybir.dt.float32
I32 = mybir.dt.int32
AF = mybir.ActivationFunctionType
OP = mybir.AluOpType
AX = mybir.AxisListType


def build_program():
    nc = bacc_mod.Bacc(None, target_bir_lowering=False, debug=False)

    tbl = nc.dram_tensor("tbl", [TBL_ROWS, ROW_I32], I32, kind="ExternalInput")
    emb = nc.dram_tensor("emb", [TBL_ROWS, DIM], FP, kind="ExternalInput")
    relt = nc.dram_tensor("relt", [DIM, NUM_REL], FP, kind="ExternalInput")
    iota = nc.dram_tensor("iota", [128, NUM_REL], FP, kind="ExternalInput")
    waug = nc.dram_tensor("w_aug", [DIM + 1, DIM], FP, kind="ExternalInput")
    seeds = nc.dram_tensor("seeds", [B_CORE], I32, kind="ExternalInput")
    out = nc.dram_tensor("out", [B_CORE, DIM], FP, kind="ExternalOutput")

    with TileContext(nc) as tc:
        with (
            tc.tile_pool(name="res", bufs=1) as res,
            tc.tile_pool(name="g1p", bufs=1) as g1p,
            tc.tile_pool(name="nvk", bufs=16) as nvk,
            tc.tile_pool(name="idxp", bufs=16) as idxp,
            tc.tile_pool(name="work", bufs=3) as work,
            tc.tile_pool(name="small", bufs=6) as small,
            tc.tile_pool(name="psum", bufs=2, space="PSUM") as psum,
        ):
            # ---- constants ----
            identity = res.tile([128, 128], FP, tag="identity")
            make_identity(nc, identity[:])
            waug_sb = res.tile([DIM + 1, DIM], FP, tag="waug")
            nc.sync.dma_start(waug_sb[:], waug[:])
            relt_sb = res.tile([DIM, NUM_REL], FP, tag="relt")
            nc.sync.dma_start(relt_sb[:], relt[:])
            iota_sb = res.tile([128, NUM_REL], FP, tag="iota")
            nc.sync.dma_start(iota_sb[:], iota[:])
            xaug = res.tile([DIM + 1, 128], FP, tag="xaug")
            nc.vector.memset(xaug[DIM : DIM + 1, :], 1.0)
            neg20 = res.tile([128, 1], FP, tag="neg20")
            nc.vector.memset(neg20[:], -20.0)
            seeds_sb = res.tile([B_CORE, 1], I32, tag="seeds")
            nc.sync.dma_start(seeds_sb[:], seeds[:].rearrange("(p o) -> p o", o=1))

            def gather_row(dest_ap, table_ap, idx01):
                nc.gpsimd.indirect_dma_start(
                    out=dest_ap,
                    out_offset=None,
                    in_=table_ap,
                    in_offset=IndirectOffsetOnAxis(ap=idx01, axis=0),
                )

            def weights_from_rels(E, rel_f32_view, tag):
                """w[p,k] = E[p, rel[p,k]] via one-hot; returns [128, NNB] tile."""
                oh = work.tile([128, NNB, NUM_REL], FP, tag="oh")
                nc.vector.tensor_tensor(
                    oh[:],
                    rel_f32_view.unsqueeze(2).to_broadcast([128, NNB, NUM_REL]),
                    iota_sb[:].unsqueeze(1).to_broadcast([128, NNB, NUM_REL]),
                    OP.is_equal,
                )
                nc.vector.tensor_tensor(
                    oh[:],
                    oh[:],
                    E.unsqueeze(1).to_broadcast([128, NNB, NUM_REL]),
                    OP.mult,
                )
                w = small.tile([128, NNB], FP, tag=tag)
                nc.vector.tensor_reduce(w[:], oh[:], AX.X, OP.add)
                return w

            def scores_E(sv_ap, tag):
                """E = exp(sv @ relT - 20): [128, 64] tile."""
                svt_ps = psum.tile([DIM, 128], FP, tag="svt_ps", space="PSUM")
                nc.tensor.transpose(svt_ps[:], sv_ap, identity[:])
                svt = small.tile([DIM, 128], FP, tag="svt")
                nc.scalar.copy(svt[:], svt_ps[:])
                s_ps = psum.tile([128, NUM_REL], FP, tag="s_ps", space="PSUM")
                nc.tensor.matmul(s_ps[:], svt[:], relt_sb[:], start=True, stop=True)
                E = small.tile([128, NUM_REL], FP, tag=tag)
                nc.scalar.activation(E[:], s_ps[:], AF.Exp, bias=neg20[:])
                return E

            def finish(agg, w, sv_ap, act_func, out_ap):
                """out = act((agg/sum(w) + sv) @ W + b)."""
                zsum = small.tile([128, 1], FP, tag="zsum")
                nc.vector.tensor_reduce(zsum[:], w[:], AX.X, OP.add)
                zinv = small.tile([128, 1], FP, tag="zinv")
                nc.vector.reciprocal(zinv[:], zsum[:])
                x = small.tile([128, DIM], FP, tag="x")
                nc.vector.scalar_tensor_tensor(
                    x[:], agg[:], zinv[:], sv_ap, OP.mult, OP.add
                )
                xt_ps = psum.tile([DIM, 128], FP, tag="xt", space="PSUM")
                nc.tensor.transpose(xt_ps[:], x[:], identity[:])
                nc.scalar.copy(xaug[0:DIM, :], xt_ps[:])
                h_ps = psum.tile([128, DIM], FP, tag="h", space="PSUM")
                nc.tensor.matmul(h_ps[:], xaug[:], waug_sb[:], start=True, stop=True)
                nc.scalar.activation(out_ap, h_ps[:], act_func)

            # ---- phase 1: seed rows ----
            g0 = res.tile([128, ROW_I32], I32, tag="g0")
            gather_row(g0[:], tbl[:], seeds_sb[:, 0:1])
            sv0 = g0[:, 0:DIM].bitcast(FP)
            r1f = res.tile([128, NNB], FP, tag="r1f")
            nc.vector.tensor_copy(r1f[:], g0[:, 2 * DIM : 3 * DIM])  # i32 -> f32

            # ---- phase 2: child rows (one fused row per drug per child) ----
            g1 = []
            for n in range(NNB):
                icol = idxp.tile([128, 1], I32, tag="icol")
                nc.vector.tensor_copy(icol[:], g0[:, DIM + n : DIM + n + 1])
                gn = g1p.tile([128, ROW_I32], I32, tag=f"g1_{n}")
                gather_row(gn[:], tbl[:], icol[:, 0:1])
                g1.append(gn)

            # ---- iter-0 hop-1: tile n = (drug p, child n); h1 -> nv1 ----
            nv1 = res.tile([128, NNB * DIM], FP, tag="nv1")
            nv1v = nv1[:].rearrange("p (n d) -> p n d", d=DIM)
            for n in range(NNB):
                gn = g1[n]
                svn = gn[:, 0:DIM].bitcast(FP)
                E = scores_E(svn, "E_t")
                r2f = small.tile([128, NNB], FP, tag="r2f")
                nc.vector.tensor_copy(r2f[:], gn[:, 2 * DIM : 3 * DIM])
                w = weights_from_rels(E[:], r2f[:], "w_t")
                agg = small.tile([128, DIM], FP, tag="agg")
                for k in range(NNB):
                    icol = idxp.tile([128, 1], I32, tag="icol")
                    nc.vector.tensor_copy(icol[:], gn[:, DIM + k : DIM + k + 1])
                    nvt = nvk.tile([128, DIM], FP, tag="nvt")
                    gather_row(nvt[:], emb[:], icol[:, 0:1])
                    if k == 0:
                        nc.vector.scalar_tensor_tensor(
                            agg[:], nvt[:], w[:, 0:1], nvt[:], OP.mult, OP.bypass
                        )
                    else:
                        nc.vector.scalar_tensor_tensor(
                            agg[:], nvt[:], w[:, k : k + 1], agg[:], OP.mult, OP.add
                        )
                ht = small.tile([128, DIM], FP, tag="ht")
                finish(agg, w, svn, AF.Sigmoid, ht[:])
                nc.vector.tensor_copy(nv1v[:, n, :], ht[:])

            # ---- iter-0 hop-0: h0 (neighbors = child emb views, rels r1) ----
            E0 = scores_E(sv0, "E_t")
            w0 = weights_from_rels(E0[:], r1f[:], "w_t")
            agg0 = res.tile([128, DIM], FP, tag="agg0")
            for k in range(NNB):
                nvt = g1[k][:, 0:DIM].bitcast(FP)
                if k == 0:
                    nc.vector.scalar_tensor_tensor(
                        agg0[:], nvt, w0[:, 0:1], nvt, OP.mult, OP.bypass
                    )
                else:
                    nc.vector.scalar_tensor_tensor(
                        agg0[:], nvt, w0[:, k : k + 1], agg0[:], OP.mult, OP.add
                    )
            h0 = res.tile([128, DIM], FP, tag="h0")
            finish(agg0, w0, sv0, AF.Sigmoid, h0[:])

            # ---- iter-1: final (neighbors = h1 in nv1, rels r1) ----
            Ef = scores_E(h0[:], "E_t")
            wf = weights_from_rels(Ef[:], r1f[:], "w_t")
            wnv = work.tile([128, NNB, DIM], FP, tag="wnv")
            nc.vector.tensor_tensor(
                wnv[:], wf[:].unsqueeze(2).to_broadcast([128, NNB, DIM]), nv1v, OP.mult
            )
            aggf = res.tile([128, DIM], FP, tag="aggf")
            nc.vector.tensor_reduce(aggf[:], wnv[:].transpose([0, 2, 1]), AX.X, OP.add)
            ofin = res.tile([128, DIM], FP, tag="ofin")
            finish(aggf, wf, h0[:], AF.Tanh, ofin[:])
            nc.sync.dma_start(out[:], ofin[:])

    nc.compile()
    return nc


_NC_CACHE = None


def _get_nc():
    global _NC_CACHE
    if _NC_CACHE is None:
        _NC_CACHE = build_program()
    return _NC_CACHE


def make_host_tables(adj_ent, adj_rel, ent_emb, rel_emb, W, b):
    ent_emb = np.asarray(ent_emb, np.float32)
    rel_emb = np.asarray(rel_emb, np.float32)
    tbl = np.zeros((TBL_ROWS, ROW_I32), dtype=np.int32)
    tbl[:NUM_ENT, 0:DIM] = ent_emb.view(np.int32)
    tbl[:NUM_ENT, DIM : 2 * DIM] = np.asarray(adj_ent).astype(np.int32)
    tbl[:NUM_ENT, 2 * DIM : 3 * DIM] = np.asarray(adj_rel).astype(np.int32)
    emb = np.zeros((TBL_ROWS, DIM), dtype=np.float32)
    emb[:NUM_ENT] = ent_emb
    emb[NUM_ENT:] = rel_emb
    relt = np.ascontiguousarray(rel_emb.T)
    iota = np.tile(np.arange(NUM_REL, dtype=np.float32), (128, 1))
    w_aug = np.concatenate(
        [np.asarray(W, np.float32), np.asarray(b, np.float32)[None, :]], axis=0
    )
    return tbl, emb, relt, iota, w_aug


def kernel(drug_entity_list, adj_ent, adj_rel, ent_emb, rel_emb, W, b, **run_kwargs):
    from concourse.bass_utils import run_bass_kernel_spmd

    nc = _get_nc()
    tbl, emb, relt, iota, w_aug = make_host_tables(
        adj_ent, adj_rel, ent_emb, rel_emb, W, b
    )
    seeds = np.asarray(drug_entity_list).astype(np.int32)
    in_maps = [
        {
            "tbl": tbl,
            "emb": emb,
            "relt": relt,
            "iota": iota,
            "w_aug": w_aug,
            "seeds": np.ascontiguousarray(seeds[c * B_CORE : (c + 1) * B_CORE]),
        }
        for c in range(N_CORES)
    ]
    res = run_bass_kernel_spmd(nc, in_maps, core_ids=list(range(N_CORES)), **run_kwargs)
    outs = [res.results[c]["out"] for c in range(N_CORES)]
    full = np.concatenate(outs, axis=0).astype(np.float32)
    kernel.last_result = res
    return full

